# revision 4
# baseline (speedup 1.0000x reference)
"""AttnGate sparse-attention block-mask kernel for 8 Trainium2 NeuronCores.

Takes the full unsharded inputs, shards batch x k-head-group across the 8
cores (core c -> batch c//2, k-heads (c%2)*4..+4), runs one SPMD Bass kernel,
and gathers the full [B, Hk, nb] boolean block mask.

Math notes (vs the reference):
  - softmax is skipped: top-k indices are invariant under a monotone map.
  - mean-pool = (1/64)*sum over the 64 positions; the 1/64 is folded into the
    mean half of Wk on the host (exact, power of two).  The sum itself is
    64 PSUM-accumulated PE transposes, which also lands the pooled tensor in
    the [d, block] layout the projection wants.
  - rmsnorm weight and the 1/sqrt(Dg) scale are folded into cos/sin on the
    host; the per-token rsqrt is applied to the final scores (rope is linear
    in x, and a per-block positive scalar commutes through it).
  - top-128 is 16 rounds of (vector.max -> match_replace with -1e30); the
    selected positions are read back with an is_equal pass.
"""

import json
import math
import os
import sys

import numpy as np

sys.path.insert(0, "/opt/trn_rl_repo")

B, S, HK, D = 4, 65536, 8, 128
BLOCK = 64
NB = S // BLOCK          # 1024 blocks
DG = 128
HQ, G = 32, 4
N_CORES = 8
HEADS_PER_CORE = HK // 2  # 4
CHUNK_BLOCKS = 128        # blocks per pipeline chunk
N_CHUNKS = NB // CHUNK_BLOCKS  # 8
POS_PER_CHUNK = CHUNK_BLOCKS * BLOCK  # 8192 tokens
NEG_MASK = -1e20
SENTINEL = -1e30
EPS = 1e-6

_compiled = {}


# ---------------------------------------------------------------------------
# walrus wait-capacity shim: split multi-wait instructions into single-wait
# NoOp carriers on the same engine (this walrus build accepts one sync wait
# per TPB instruction struct on the failing paths).
# ---------------------------------------------------------------------------
def _split_waits_json(bir_json):
    j = json.loads(bir_json.decode() if isinstance(bir_json, (bytes, bytearray)) else bir_json)
    n = 0
    for f in j.get("functions", []):
        for blk in f.get("blocks", []):
            out = []
            for inst in blk.get("instructions", []):
                si = inst.get("sync_info")
                waits = si.get("on_wait", []) if si else []
                if len(waits) > 1 and inst.get("engine") not in (None, "Unassigned"):
                    for w in waits[:-1]:
                        n += 1
                        out.append({
                            "debug": inst.get("debug", 0),
                            "engine": inst["engine"],
                            "ins": [], "outs": [],
                            "name": "WC-%d" % n,
                            "opcode": "NoOp",
                            "sync_info": {"on_update": [], "on_wait": [w]},
                        })
                    si["on_wait"] = waits[-1:]
                out.append(inst)
            blk["instructions"] = out
    return json.dumps(j).encode()


def _install_waitfix():
    import concourse.bass_utils as bu
    import concourse.bass2jax as b2j
    if getattr(bu, "_attngate_waitfix", False):
        return
    orig = bu.compile_bir_kernel

    def patched(bir_json, tmpdir, neff_name="file.neff"):
        return orig(_split_waits_json(bir_json), tmpdir, neff_name)

    bu.compile_bir_kernel = patched
    b2j.compile_bir_kernel = patched
    bu._attngate_waitfix = True


# ---------------------------------------------------------------------------
# device program
# ---------------------------------------------------------------------------
def _build_program(n_rounds):
    import concourse.bass as bass
    import concourse.mybir as mybir
    from concourse import tile
    from contextlib import ExitStack

    dt = mybir.dt
    f32 = dt.float32
    AX = mybir.AxisListType
    ALU = mybir.AluOpType

    nc = bass.Bass()

    k_d = [nc.dram_tensor("k%d" % h, [S, D], f32, kind="ExternalInput")
           for h in range(HEADS_PER_CORE)]
    wk_d = nc.dram_tensor("wk", [D, HEADS_PER_CORE, 2, DG], f32, kind="ExternalInput")
    wq_d = nc.dram_tensor("wq", [D, HEADS_PER_CORE * G * DG], f32, kind="ExternalInput")
    qv_d = nc.dram_tensor("qvec", [D, HEADS_PER_CORE * G], f32, kind="ExternalInput")
    cq_d = nc.dram_tensor("cq", [HEADS_PER_CORE, DG], f32, kind="ExternalInput")
    sq_d = nc.dram_tensor("sq", [HEADS_PER_CORE, DG], f32, kind="ExternalInput")
    ck_d = nc.dram_tensor("ck", [NB, DG], f32, kind="ExternalInput")
    sk_d = nc.dram_tensor("sk", [NB, DG], f32, kind="ExternalInput")
    am_d = nc.dram_tensor("amask", [HEADS_PER_CORE, NB], f32, kind="ExternalInput")
    idn_d = nc.dram_tensor("idn", [128, 128], f32, kind="ExternalInput")
    ones_d = nc.dram_tensor("ones_col", [128, 1], f32, kind="ExternalInput")
    out_d = nc.dram_tensor("out_mask", [HEADS_PER_CORE, NB], f32, kind="ExternalOutput")

    with tile.TileContext(nc) as tc, ExitStack() as ctx:
        consts = ctx.enter_context(tc.tile_pool(name="consts", bufs=1))
        chunks = ctx.enter_context(tc.tile_pool(name="chunks", bufs=2))
        stores = ctx.enter_context(tc.tile_pool(name="stores", bufs=2))
        small = ctx.enter_context(tc.tile_pool(name="small", bufs=2))
        psA_p = ctx.enter_context(tc.tile_pool(name="psA", bufs=2, space="PSUM"))
        psT_p = ctx.enter_context(tc.tile_pool(name="psT", bufs=2, space="PSUM"))
        psC_p = ctx.enter_context(tc.tile_pool(name="psC", bufs=2, space="PSUM"))
        psS_p = ctx.enter_context(tc.tile_pool(name="psS", bufs=2, space="PSUM"))

        # ---- constants / small inputs -----------------------------------
        idn = consts.tile([128, 128], f32)
        nc.scalar.dma_start(idn[:], idn_d[:, :])
        ones = consts.tile([128, 1], f32)
        nc.scalar.dma_start(ones[:], ones_d[:, :])
        wk = consts.tile([128, HEADS_PER_CORE * 2 * DG], f32)
        nc.scalar.dma_start(wk[:], wk_d[:, :, :, :].rearrange("d h t o -> d (h t o)"))
        wq = consts.tile([128, HEADS_PER_CORE * G * DG], f32)
        nc.scalar.dma_start(wq[:], wq_d[:, :])
        qvec = consts.tile([128, HEADS_PER_CORE * G], f32)
        nc.scalar.dma_start(qvec[:], qv_d[:, :])
        cq = consts.tile([HEADS_PER_CORE, DG], f32)
        nc.scalar.dma_start(cq[:], cq_d[:, :])
        sq = consts.tile([HEADS_PER_CORE, DG], f32)
        nc.scalar.dma_start(sq[:], sq_d[:, :])
        amask = consts.tile([HEADS_PER_CORE, NB], f32)
        nc.scalar.dma_start(amask[:], am_d[:, :])

        # cos_k / sin_k arrive [block, o]; transpose to [o, block] via PE.
        ckT = consts.tile([128, NB], f32)
        skT = consts.tile([128, NB], f32)
        for src_d, dstT in ((ck_d, ckT), (sk_d, skT)):
            stage = small.tile([128, 8 * 128], f32, tag="cs_stage")
            nc.scalar.dma_start(
                stage[:], src_d[:, :].rearrange("(j p) o -> p j o", p=128))
            for j in range(8):
                pst = psT_p.tile([128, 128], f32, tag="psT")
                nc.tensor.matmul(pst[:], stage[:, j * 128:(j + 1) * 128], idn[:],
                                 is_transpose=True, start=True, stop=True)
                nc.scalar.copy(dstT[:, j * 128:(j + 1) * 128], pst[:])

        # ---- q path ------------------------------------------------------
        # qp[h] = sum_j qvec_chunk_j.T @ Wq_chunk_j  -> [1, DG] rows
        qp = small.tile([HEADS_PER_CORE, DG], f32, tag="qp")
        for h in range(HEADS_PER_CORE):
            psq = psS_p.tile([1, DG], f32, tag="psS")
            for j in range(G):
                nc.tensor.matmul(
                    psq[:], qvec[:, h * G + j:h * G + j + 1],
                    wq[:, (h * G + j) * DG:(h * G + j + 1) * DG],
                    start=(j == 0), stop=(j == G - 1))
            qstage = small.tile([1, DG], f32, tag="qstage")
            nc.scalar.copy(qstage[:], psq[:])
            nc.scalar.dma_start(qp[h:h + 1, :], qstage[:])
        # rmsnorm (weight folded into cq/sq on host)
        qsqr = small.tile([HEADS_PER_CORE, DG], f32, tag="qsqr")
        nc.vector.tensor_tensor(qsqr[:], qp[:], qp[:], ALU.mult)
        qss = small.tile([HEADS_PER_CORE, 1], f32, tag="qss")
        nc.vector.tensor_reduce(qss[:], qsqr[:], axis=AX.X, op=ALU.add)
        nc.vector.tensor_scalar(qss[:], qss[:], 1.0 / DG, EPS, ALU.mult, ALU.add)
        nc.vector.reciprocal(qss[:], qss[:])
        nc.scalar.activation(qss[:], qss[:], mybir.ActivationFunctionType.Sqrt)
        qn = small.tile([HEADS_PER_CORE, DG], f32, tag="qn")
        nc.vector.tensor_scalar(qn[:], qp[:], qss[:], None, ALU.mult)
        # rope: qv = qn*cq + rot_half(qn)*sq   (cq/sq carry w, sign and scale)
        qv1 = small.tile([HEADS_PER_CORE, DG], f32, tag="qv1")
        nc.vector.tensor_tensor(qv1[:], qn[:], cq[:], ALU.mult)
        qv2 = small.tile([HEADS_PER_CORE, DG], f32, tag="qv2")
        nc.vector.tensor_tensor(qv2[:, 0:64], qn[:, 64:128], sq[:, 0:64], ALU.mult)
        nc.vector.tensor_tensor(qv2[:, 64:128], qn[:, 0:64], sq[:, 64:128], ALU.mult)
        nc.vector.tensor_tensor(qv1[:], qv1[:], qv2[:], ALU.add)
        # transpose to [o, h] for the score matmuls
        psqt = psT_p.tile([128, 128], f32, tag="psT")
        nc.tensor.matmul(psqt[0:DG, 0:HEADS_PER_CORE], qv1[:],
                         idn[0:HEADS_PER_CORE, 0:HEADS_PER_CORE],
                         is_transpose=True, start=True, stop=True)
        qvT = small.tile([128, HEADS_PER_CORE], f32, tag="qvT")
        nc.scalar.copy(qvT[:], psqt[0:128, 0:HEADS_PER_CORE])

        # score accumulator [h, NB]
        sc_all = consts.tile([HEADS_PER_CORE, NB], f32)
        rs_all = consts.tile([HEADS_PER_CORE, NB], f32)

        # ---- main loop ---------------------------------------------------
        for h in range(HEADS_PER_CORE):
            meanT = stores.tile([128, NB], f32, tag="meanT")
            maxT = stores.tile([128, NB], f32, tag="maxT")
            for c in range(N_CHUNKS):
                kt = chunks.tile([128, POS_PER_CHUNK], f32, tag="kt")
                nc.scalar.dma_start(
                    kt[:],
                    k_d[h][c * POS_PER_CHUNK:(c + 1) * POS_PER_CHUNK, :]
                    .rearrange("(p f) d -> p (f d)", p=128))
                # mean: 64 accumulated fp32 transposes -> [d, blk]
                psA = psA_p.tile([128, CHUNK_BLOCKS], f32, tag="psA")
                for p in range(BLOCK):
                    nc.tensor.matmul(psA[:], kt[:, p * D:(p + 1) * D], idn[:],
                                     is_transpose=True,
                                     start=(p == 0), stop=(p == BLOCK - 1))
                nc.scalar.copy(meanT[:, c * CHUNK_BLOCKS:(c + 1) * CHUNK_BLOCKS], psA[:])
                # max: windowed reduce over pos (innermost, strided)
                mstage = small.tile([128, D], f32, tag="mstage")
                nc.vector.tensor_reduce(
                    mstage[:],
                    kt[:, :].rearrange("p (a b) -> p b a", a=BLOCK, b=D),
                    axis=AX.X, op=ALU.max, opt_input=False)
                psM = psT_p.tile([128, 128], f32, tag="psT")
                nc.tensor.matmul(psM[:], mstage[:], idn[:],
                                 is_transpose=True, start=True, stop=True)
                nc.scalar.copy(maxT[:, c * CHUNK_BLOCKS:(c + 1) * CHUNK_BLOCKS], psM[:])

            # ---- phase 2 for this head ----------------------------------
            kcT = stores.tile([128, NB], f32, tag="kcT")
            for g in range(2):
                sl = slice(g * 512, (g + 1) * 512)
                psC = psC_p.tile([128, 512], f32, tag="psC")
                nc.tensor.matmul(psC[:], wk[:, (h * 2) * DG:(h * 2 + 1) * DG],
                                 meanT[:, sl], start=True, stop=False)
                nc.tensor.matmul(psC[:], wk[:, (h * 2 + 1) * DG:(h * 2 + 2) * DG],
                                 maxT[:, sl], start=False, stop=True)
                nc.scalar.copy(kcT[:, sl], psC[:])

            # rms inverse scale (applied later to the scores)
            kcsq = stores.tile([128, NB], f32, tag="kcsq")
            nc.vector.tensor_tensor(kcsq[:], kcT[:], kcT[:], ALU.mult)
            rstage = small.tile([1, NB], f32, tag="rstage")
            for g in range(2):
                sl = slice(g * 512, (g + 1) * 512)
                psR = psS_p.tile([1, 512], f32, tag="psS")
                nc.tensor.matmul(psR[:], ones[:], kcsq[:, sl], start=True, stop=True)
                nc.scalar.copy(rstage[:, sl], psR[:])
            nc.vector.tensor_scalar(rstage[:], rstage[:],
                                    1.0 / DG, EPS, ALU.mult, ALU.add)
            nc.vector.reciprocal(rstage[:], rstage[:])
            nc.scalar.activation(rstage[:], rstage[:],
                                 mybir.ActivationFunctionType.Sqrt)
            nc.scalar.dma_start(rs_all[h:h + 1, :], rstage[:])

            # rope on kcT: rope = kcT*ckT + rot_half(kcT)*skT
            rp1 = stores.tile([128, NB], f32, tag="rp1")
            nc.vector.tensor_tensor(rp1[:], kcT[:], ckT[:], ALU.mult)
            kcrot = stores.tile([128, NB], f32, tag="kcrot")
            nc.scalar.dma_start(kcrot[0:64, :], kcT[64:128, :])
            nc.scalar.dma_start(kcrot[64:128, :], kcT[0:64, :])
            rp2 = stores.tile([128, NB], f32, tag="rp2")
            nc.vector.tensor_tensor(rp2[:], kcrot[:], skT[:], ALU.mult)
            nc.vector.tensor_tensor(rp1[:], rp1[:], rp2[:], ALU.add)

            # scores: qvT[:, h].T @ rope  -> [1, NB]
            scstage = small.tile([1, NB], f32, tag="scstage")
            for g in range(2):
                sl = slice(g * 512, (g + 1) * 512)
                psSc = psS_p.tile([1, 512], f32, tag="psS")
                nc.tensor.matmul(psSc[:], qvT[:, h:h + 1], rp1[:, sl],
                                 start=True, stop=True)
                nc.scalar.copy(scstage[:, sl], psSc[:])
            nc.scalar.dma_start(sc_all[h:h + 1, :], scstage[:])

        # ---- epilogue: norm-scale, mask, topk ---------------------------
        nc.vector.tensor_tensor(sc_all[:], sc_all[:], rs_all[:], ALU.mult)
        mterm = small.tile([HEADS_PER_CORE, NB], f32, tag="mterm")
        nc.vector.tensor_scalar(mterm[:], amask[:], -NEG_MASK, NEG_MASK,
                                ALU.mult, ALU.add)
        nc.vector.tensor_tensor(sc_all[:], sc_all[:], amask[:], ALU.mult)
        nc.vector.tensor_tensor(sc_all[:], sc_all[:], mterm[:], ALU.add)
        m8 = small.tile([HEADS_PER_CORE, 8], f32, tag="m8")
        for _ in range(n_rounds):
            nc.vector.max(m8[:], sc_all[:])
            nc.vector.match_replace(sc_all[:], m8[:], sc_all[:], SENTINEL)
        sel = small.tile([HEADS_PER_CORE, NB], f32, tag="sel")
        nc.vector.tensor_scalar(sel[:], sc_all[:], SENTINEL, None, ALU.is_equal)
        nc.scalar.dma_start(out_d[:, :], sel[:])

    return nc


def _rot_w(w):
    return np.concatenate([w[DG // 2:], w[:DG // 2]])


def kernel(k, q, Wq, Wk, qnorm_w, knorm_w, cos_q, sin_q, cos_k, sin_k,
           attention_mask, block_budget):
    _install_waitfix()
    from concourse.bass_utils import run_bass_kernel_spmd

    k = np.asarray(k, dtype=np.float32)
    q = np.asarray(q, dtype=np.float32)
    Wq = np.asarray(Wq, dtype=np.float32)
    Wk = np.asarray(Wk, dtype=np.float32)
    qnorm_w = np.asarray(qnorm_w, dtype=np.float32)
    knorm_w = np.asarray(knorm_w, dtype=np.float32)
    cos_q = np.asarray(cos_q, dtype=np.float32)
    sin_q = np.asarray(sin_q, dtype=np.float32)
    cos_k = np.asarray(cos_k, dtype=np.float32)
    sin_k = np.asarray(sin_k, dtype=np.float32)
    am = np.asarray(attention_mask).astype(bool)
    budget = int(block_budget)
    assert budget % 8 == 0 and 0 < budget <= NB
    n_rounds = budget // 8

    scale = 1.0 / math.sqrt(DG)

    key = (n_rounds,)
    if key not in _compiled:
        _compiled[key] = _build_program(n_rounds)
    nc = _compiled[key]

    idn_np = np.eye(128, dtype=np.float32)
    ones_np = np.ones((128, 1), dtype=np.float32)

    in_maps = []
    for c in range(N_CORES):
        b = c // 2
        h0 = (c % 2) * HEADS_PER_CORE
        heads = list(range(h0, h0 + HEADS_PER_CORE))
        im = {}
        for i, h in enumerate(heads):
            im["k%d" % i] = np.ascontiguousarray(k[b, :, h, :])
        # wk: [d, h, t(mean/max), o]; mean part scaled by 1/64
        wk_prep = np.empty((D, HEADS_PER_CORE, 2, DG), dtype=np.float32)
        for i, h in enumerate(heads):
            wk_prep[:, i, 0, :] = Wk[h, :D, :] / BLOCK
            wk_prep[:, i, 1, :] = Wk[h, D:, :]
        im["wk"] = wk_prep
        # wq: [d, (h, g, o)] with contraction index i=(g,d) split as d-partition
        wq_prep = np.empty((D, HEADS_PER_CORE, G, DG), dtype=np.float32)
        for i, h in enumerate(heads):
            wq_prep[:, i, :, :] = Wq[h].reshape(G, D, DG).transpose(1, 0, 2)
        im["wq"] = wq_prep.reshape(D, HEADS_PER_CORE * G * DG)
        # qvec: [d, (h, g)]
        qv_prep = np.empty((D, HEADS_PER_CORE, G), dtype=np.float32)
        for i, h in enumerate(heads):
            qv_prep[:, i, :] = q[b, 0, h * G:(h + 1) * G, :].T
        im["qvec"] = qv_prep.reshape(D, HEADS_PER_CORE * G)
        # folded cos/sin (q): carry qnorm_w, rotation sign and the 1/sqrt(Dg)
        cqv = cos_q[b, 0] * qnorm_w * scale
        sqv = sin_q[b, 0] * _rot_w(qnorm_w) * scale
        sqv = sqv.copy()
        sqv[:DG // 2] *= -1.0
        im["cq"] = np.tile(cqv, (HEADS_PER_CORE, 1)).astype(np.float32)
        im["sq"] = np.tile(sqv, (HEADS_PER_CORE, 1)).astype(np.float32)
        # folded cos/sin (k)
        ckv = cos_k[b] * knorm_w[None, :]
        skv = sin_k[b] * _rot_w(knorm_w)[None, :]
        skv = skv.copy()
        skv[:, :DG // 2] *= -1.0
        im["ck"] = ckv.astype(np.float32)
        im["sk"] = skv.astype(np.float32)
        im["amask"] = am[b, heads, :].astype(np.float32)
        im["idn"] = idn_np
        im["ones_col"] = ones_np
        in_maps.append(im)

    res = run_bass_kernel_spmd(nc, in_maps, core_ids=list(range(N_CORES)),
                               trace=bool(int(os.environ.get("ATTNGATE_TRACE", "0"))))
    kernel.last_result = res

    sel = np.zeros((B, HK, NB), dtype=bool)
    for c in range(N_CORES):
        b = c // 2
        h0 = (c % 2) * HEADS_PER_CORE
        sel[b, h0:h0 + HEADS_PER_CORE, :] = res.results[c]["out_mask"] != 0.0
    mask = sel & am
    mask[:, :, -1] = True
    return mask


# revision 12
# speedup vs baseline: 1.1668x; 1.1668x over previous
"""AttnGate sparse-attention block-mask kernel for 8 Trainium2 NeuronCores.

Takes the full unsharded inputs, shards batch x k-head-group across the 8
cores (core c -> batch c//2, k-heads (c%2)*4..+4), runs one SPMD Bass kernel,
and gathers the full [B, Hk, nb] boolean block mask.

Math notes (vs the reference):
  - softmax is skipped: top-k indices are invariant under a monotone map.
  - mean-pool = (1/64)*sum over the 64 positions; the 1/64 is folded into the
    mean half of Wk on the host (exact, power of two).  The sum itself is
    64 PSUM-accumulated PE transposes, which also lands the pooled tensor in
    the [d, block] layout the projection wants.
  - rmsnorm weight and the 1/sqrt(Dg) scale are folded into cos/sin on the
    host; the per-token rsqrt is applied to the final scores (rope is linear
    in x, and a per-block positive scalar commutes through it).
  - top-128 is 16 rounds of (vector.max -> match_replace with -1e30); the
    selected positions are read back with an is_equal pass.
"""

import json
import math
import os
import sys

import numpy as np

sys.path.insert(0, "/opt/trn_rl_repo")

B, S, HK, D = 4, 65536, 8, 128
BLOCK = 64
NB = S // BLOCK          # 1024 blocks
DG = 128
HQ, G = 32, 4
N_CORES = 8
HEADS_PER_CORE = HK // 2  # 4
CHUNK_BLOCKS = 128        # blocks per pipeline chunk
N_CHUNKS = NB // CHUNK_BLOCKS  # 8
POS_PER_CHUNK = CHUNK_BLOCKS * BLOCK  # 8192 tokens
NEG_MASK = -1e20
SENTINEL = -1e30
EPS = 1e-6

_compiled = {}


# ---------------------------------------------------------------------------
# walrus wait-capacity shim: split multi-wait instructions into single-wait
# NoOp carriers on the same engine (this walrus build accepts one sync wait
# per TPB instruction struct on the failing paths).
# ---------------------------------------------------------------------------
def _split_waits_json(bir_json):
    j = json.loads(bir_json.decode() if isinstance(bir_json, (bytes, bytearray)) else bir_json)
    n = 0
    for f in j.get("functions", []):
        for blk in f.get("blocks", []):
            out = []
            for inst in blk.get("instructions", []):
                si = inst.get("sync_info")
                waits = si.get("on_wait", []) if si else []
                if len(waits) > 1 and inst.get("engine") not in (None, "Unassigned"):
                    for w in waits[:-1]:
                        n += 1
                        out.append({
                            "debug": inst.get("debug", 0),
                            "engine": inst["engine"],
                            "ins": [], "outs": [],
                            "name": "WC-%d" % n,
                            "opcode": "NoOp",
                            "sync_info": {"on_update": [], "on_wait": [w]},
                        })
                    si["on_wait"] = waits[-1:]
                out.append(inst)
            blk["instructions"] = out
    return json.dumps(j).encode()


def _install_waitfix():
    import concourse.bass_utils as bu
    import concourse.bass2jax as b2j
    if getattr(bu, "_attngate_waitfix", False):
        return
    orig = bu.compile_bir_kernel

    def patched(bir_json, tmpdir, neff_name="file.neff"):
        return orig(_split_waits_json(bir_json), tmpdir, neff_name)

    bu.compile_bir_kernel = patched
    b2j.compile_bir_kernel = patched
    bu._attngate_waitfix = True


# ---------------------------------------------------------------------------
# device program
# ---------------------------------------------------------------------------
def _build_program(n_rounds):
    import concourse.bass as bass
    import concourse.mybir as mybir
    from concourse import tile
    from contextlib import ExitStack

    dt = mybir.dt
    f32 = dt.float32
    AX = mybir.AxisListType
    ALU = mybir.AluOpType

    nc = bass.Bass()

    k_d = [nc.dram_tensor("k%d" % h, [S, D], f32, kind="ExternalInput")
           for h in range(HEADS_PER_CORE)]
    wk_d = nc.dram_tensor("wk", [D, HEADS_PER_CORE, 2, DG], f32, kind="ExternalInput")
    wq_d = nc.dram_tensor("wq", [D, HEADS_PER_CORE * G * DG], f32, kind="ExternalInput")
    qv_d = nc.dram_tensor("qvec", [D, HEADS_PER_CORE * G], f32, kind="ExternalInput")
    cq_d = nc.dram_tensor("cq", [HEADS_PER_CORE, DG], f32, kind="ExternalInput")
    sq_d = nc.dram_tensor("sq", [HEADS_PER_CORE, DG], f32, kind="ExternalInput")
    ck_d = nc.dram_tensor("ck", [NB, DG], f32, kind="ExternalInput")
    sk_d = nc.dram_tensor("sk", [NB, DG], f32, kind="ExternalInput")
    am_d = nc.dram_tensor("amask", [HEADS_PER_CORE, NB], f32, kind="ExternalInput")
    idn_d = nc.dram_tensor("idn", [128, 128], f32, kind="ExternalInput")
    ones_d = nc.dram_tensor("ones_col", [128, 1], f32, kind="ExternalInput")
    out_d = nc.dram_tensor("out_mask", [HEADS_PER_CORE, NB], f32, kind="ExternalOutput")

    with tile.TileContext(nc) as tc, ExitStack() as ctx:
        consts = ctx.enter_context(tc.tile_pool(name="consts", bufs=1))
        chunks = ctx.enter_context(tc.tile_pool(name="chunks", bufs=2))
        stores = ctx.enter_context(tc.tile_pool(name="stores", bufs=2))
        small = ctx.enter_context(tc.tile_pool(name="small", bufs=1))
        trees = ctx.enter_context(tc.tile_pool(name="trees", bufs=2))
        psA_p = ctx.enter_context(tc.tile_pool(name="psA", bufs=2, space="PSUM"))
        psT_p = ctx.enter_context(tc.tile_pool(name="psT", bufs=2, space="PSUM"))
        psC_p = ctx.enter_context(tc.tile_pool(name="psC", bufs=2, space="PSUM"))
        psS_p = ctx.enter_context(tc.tile_pool(name="psS", bufs=2, space="PSUM"))

        # ---- constants / small inputs -----------------------------------
        idn = consts.tile([128, 128], f32)
        nc.gpsimd.dma_start(idn[:], idn_d[:, :])
        ones = consts.tile([128, 1], f32)
        nc.gpsimd.dma_start(ones[:], ones_d[:, :])
        wk = consts.tile([128, HEADS_PER_CORE * 2 * DG], f32)
        nc.gpsimd.dma_start(wk[:], wk_d[:, :, :, :].rearrange("d h t o -> d (h t o)"))
        wq = stores1.tile([128, HEADS_PER_CORE * G * DG], f32, tag="rp1")
        nc.gpsimd.dma_start(wq[:], wq_d[:, :])
        qvec = consts.tile([128, HEADS_PER_CORE * G], f32)
        nc.gpsimd.dma_start(qvec[:], qv_d[:, :])
        cq = consts.tile([HEADS_PER_CORE, DG], f32)
        nc.gpsimd.dma_start(cq[:], cq_d[:, :])
        sq = consts.tile([HEADS_PER_CORE, DG], f32)
        nc.gpsimd.dma_start(sq[:], sq_d[:, :])
        amask = consts.tile([HEADS_PER_CORE, NB], f32)
        nc.gpsimd.dma_start(amask[:], am_d[:, :])

        # cos_k / sin_k arrive [block, o]; transpose to [o, block] via PE.
        ckT = consts.tile([128, NB], f32)
        skT = consts.tile([128, NB], f32)
        for src_d, dstT in ((ck_d, ckT), (sk_d, skT)):
            stage = small.tile([128, 8 * 128], f32, tag="cs_stage")
            nc.gpsimd.dma_start(
                stage[:], src_d[:, :].rearrange("(j p) o -> p j o", p=128))
            for j in range(8):
                pst = psT_p.tile([128, 128], f32, tag="psT")
                nc.tensor.matmul(pst[:], stage[:, j * 128:(j + 1) * 128], idn[:],
                                 is_transpose=True, start=True, stop=True)
                nc.scalar.copy(dstT[:, j * 128:(j + 1) * 128], pst[:])

        # ---- q path ------------------------------------------------------
        # qp[h] = sum_j qvec_chunk_j.T @ Wq_chunk_j  -> [1, DG] rows
        qp = small.tile([HEADS_PER_CORE, DG], f32, tag="qp")
        for h in range(HEADS_PER_CORE):
            psq = psS_p.tile([1, DG], f32, tag="psS")
            for j in range(G):
                nc.tensor.matmul(
                    psq[:], qvec[:, h * G + j:h * G + j + 1],
                    wq[:, (h * G + j) * DG:(h * G + j + 1) * DG],
                    start=(j == 0), stop=(j == G - 1))
            qstage = small.tile([1, DG], f32, tag="qstage")
            nc.scalar.copy(qstage[:], psq[:])
            nc.scalar.dma_start(qp[h:h + 1, :], qstage[:])
        # rmsnorm (weight folded into cq/sq on host)
        qsqr = small.tile([HEADS_PER_CORE, DG], f32, tag="qsqr")
        nc.vector.tensor_tensor(qsqr[:], qp[:], qp[:], ALU.mult)
        qss = small.tile([HEADS_PER_CORE, 1], f32, tag="qss")
        nc.vector.tensor_reduce(qss[:], qsqr[:], axis=AX.X, op=ALU.add)
        nc.vector.tensor_scalar(qss[:], qss[:], 1.0 / DG, EPS, ALU.mult, ALU.add)
        nc.vector.reciprocal(qss[:], qss[:])
        nc.scalar.activation(qss[:], qss[:], mybir.ActivationFunctionType.Sqrt)
        qn = small.tile([HEADS_PER_CORE, DG], f32, tag="qn")
        nc.vector.tensor_scalar(qn[:], qp[:], qss[:], None, ALU.mult)
        # rope: qv = qn*cq + rot_half(qn)*sq   (cq/sq carry w, sign and scale)
        qv1 = small.tile([HEADS_PER_CORE, DG], f32, tag="qv1")
        nc.vector.tensor_tensor(qv1[:], qn[:], cq[:], ALU.mult)
        qv2 = small.tile([HEADS_PER_CORE, DG], f32, tag="qv2")
        nc.vector.tensor_tensor(qv2[:, 0:64], qn[:, 64:128], sq[:, 0:64], ALU.mult)
        nc.vector.tensor_tensor(qv2[:, 64:128], qn[:, 0:64], sq[:, 64:128], ALU.mult)
        nc.vector.tensor_tensor(qv1[:], qv1[:], qv2[:], ALU.add)
        # transpose to [o, h] for the score matmuls
        psqt = psT_p.tile([128, 128], f32, tag="psT")
        nc.tensor.matmul(psqt[0:DG, 0:HEADS_PER_CORE], qv1[:],
                         idn[0:HEADS_PER_CORE, 0:HEADS_PER_CORE],
                         is_transpose=True, start=True, stop=True)
        qvT = small.tile([128, HEADS_PER_CORE], f32, tag="qvT")
        nc.scalar.copy(qvT[:], psqt[0:128, 0:HEADS_PER_CORE])

        # score accumulator [h, NB]
        sc_all = consts.tile([HEADS_PER_CORE, NB], f32)
        rs_all = consts.tile([HEADS_PER_CORE, NB], f32)

        # ---- main loop ---------------------------------------------------
        for h in range(HEADS_PER_CORE):
            meanT = stores.tile([128, NB], f32, tag="meanT")
            maxT = stores.tile([128, NB], f32, tag="maxT")
            for c in range(N_CHUNKS):
                kt = chunks.tile([128, POS_PER_CHUNK], f32, tag="kt")
                nc.scalar.dma_start(
                    kt[:],
                    k_d[h][c * POS_PER_CHUNK:(c + 1) * POS_PER_CHUNK, :]
                    .rearrange("(p f) d -> p (f d)", p=128))
                # mean: 64 accumulated fp32 transposes -> [d, blk]
                psA = psA_p.tile([128, CHUNK_BLOCKS], f32, tag="psA")
                for p in range(BLOCK):
                    nc.tensor.matmul(psA[:], kt[:, p * D:(p + 1) * D], idn[:],
                                     is_transpose=True,
                                     start=(p == 0), stop=(p == BLOCK - 1))
                nc.scalar.copy(meanT[:, c * CHUNK_BLOCKS:(c + 1) * CHUNK_BLOCKS], psA[:])
                # max: contiguous pairwise tree; first halving on GpSimd
                tr = trees.tile([128, 4096], f32, tag="tree")
                nc.vector.tensor_tensor(tr[:], kt[:, 0:4096], kt[:, 4096:8192],
                                        ALU.max)
                prev, size = tr, 4096
                while size > D:
                    half = size // 2
                    nxt = trees.tile([128, half], f32, tag="tree")
                    nc.vector.tensor_tensor(nxt[:], prev[:, 0:half],
                                            prev[:, half:size], ALU.max)
                    prev, size = nxt, half
                psM = psT_p.tile([128, 128], f32, tag="psT")
                nc.tensor.matmul(psM[:], prev[:], idn[:],
                                 is_transpose=True, start=True, stop=True)
                nc.scalar.copy(maxT[:, c * CHUNK_BLOCKS:(c + 1) * CHUNK_BLOCKS], psM[:])

            # ---- phase 2 for this head ----------------------------------
            kcT = stores.tile([128, NB], f32, tag="kcT")
            for g in range(2):
                sl = slice(g * 512, (g + 1) * 512)
                psC = psC_p.tile([128, 512], f32, tag="psC")
                nc.tensor.matmul(psC[:], wk[:, (h * 2) * DG:(h * 2 + 1) * DG],
                                 meanT[:, sl], start=True, stop=False)
                nc.tensor.matmul(psC[:], wk[:, (h * 2 + 1) * DG:(h * 2 + 2) * DG],
                                 maxT[:, sl], start=False, stop=True)
                nc.scalar.copy(kcT[:, sl], psC[:])

            # rms inverse scale (applied later to the scores)
            kcsq = stores.tile([128, NB], f32, tag="kcsq")
            nc.vector.tensor_tensor(kcsq[:], kcT[:], kcT[:], ALU.mult)
            rstage = small.tile([1, NB], f32, tag="rstage")
            for g in range(2):
                sl = slice(g * 512, (g + 1) * 512)
                psR = psS_p.tile([1, 512], f32, tag="psS")
                nc.tensor.matmul(psR[:], ones[:], kcsq[:, sl], start=True, stop=True)
                nc.scalar.copy(rstage[:, sl], psR[:])
            nc.scalar.dma_start(rs_all[h:h + 1, :], rstage[:])

            # rope on kcT: rope = kcT*ckT + rot_half(kcT)*skT
            rp1 = stores.tile([128, NB], f32, tag="rp1")
            nc.vector.tensor_tensor(rp1[:], kcT[:], ckT[:], ALU.mult)
            kcrot = stores.tile([128, NB], f32, tag="kcrot")
            nc.scalar.dma_start(kcrot[0:64, :], kcT[64:128, :])
            nc.scalar.dma_start(kcrot[64:128, :], kcT[0:64, :])
            rp2 = stores.tile([128, NB], f32, tag="rp2")
            nc.vector.tensor_tensor(rp2[:], kcrot[:], skT[:], ALU.mult)
            nc.vector.tensor_tensor(rp1[:], rp1[:], rp2[:], ALU.add)

            # scores: qvT[:, h].T @ rope  -> [1, NB]
            scstage = small.tile([1, NB], f32, tag="scstage")
            for g in range(2):
                sl = slice(g * 512, (g + 1) * 512)
                psSc = psS_p.tile([1, 512], f32, tag="psS")
                nc.tensor.matmul(psSc[:], qvT[:, h:h + 1], rp1[:, sl],
                                 start=True, stop=True)
                nc.scalar.copy(scstage[:, sl], psSc[:])
            nc.scalar.dma_start(sc_all[h:h + 1, :], scstage[:])

        # ---- epilogue: norm-scale, mask, topk ---------------------------
        nc.vector.tensor_scalar(rs_all[:], rs_all[:], 1.0 / DG, EPS,
                                ALU.mult, ALU.add)
        nc.vector.reciprocal(rs_all[:], rs_all[:])
        nc.scalar.activation(rs_all[:], rs_all[:],
                             mybir.ActivationFunctionType.Sqrt)
        nc.vector.tensor_tensor(sc_all[:], sc_all[:], rs_all[:], ALU.mult)
        mterm = small.tile([HEADS_PER_CORE, NB], f32, tag="mterm")
        nc.vector.tensor_scalar(mterm[:], amask[:], -NEG_MASK, NEG_MASK,
                                ALU.mult, ALU.add)
        nc.vector.tensor_tensor(sc_all[:], sc_all[:], amask[:], ALU.mult)
        nc.vector.tensor_tensor(sc_all[:], sc_all[:], mterm[:], ALU.add)
        m8 = small.tile([HEADS_PER_CORE, 8], f32, tag="m8")
        for _ in range(n_rounds):
            nc.vector.max(m8[:], sc_all[:])
            nc.vector.match_replace(sc_all[:], m8[:], sc_all[:], SENTINEL)
        sel = small.tile([HEADS_PER_CORE, NB], f32, tag="sel")
        nc.vector.tensor_scalar(sel[:], sc_all[:], SENTINEL, None, ALU.is_equal)
        nc.scalar.dma_start(out_d[:, :], sel[:])

    return nc


def _rot_w(w):
    return np.concatenate([w[DG // 2:], w[:DG // 2]])


def kernel(k, q, Wq, Wk, qnorm_w, knorm_w, cos_q, sin_q, cos_k, sin_k,
           attention_mask, block_budget):
    _install_waitfix()
    from concourse.bass_utils import run_bass_kernel_spmd

    k = np.asarray(k, dtype=np.float32)
    q = np.asarray(q, dtype=np.float32)
    Wq = np.asarray(Wq, dtype=np.float32)
    Wk = np.asarray(Wk, dtype=np.float32)
    qnorm_w = np.asarray(qnorm_w, dtype=np.float32)
    knorm_w = np.asarray(knorm_w, dtype=np.float32)
    cos_q = np.asarray(cos_q, dtype=np.float32)
    sin_q = np.asarray(sin_q, dtype=np.float32)
    cos_k = np.asarray(cos_k, dtype=np.float32)
    sin_k = np.asarray(sin_k, dtype=np.float32)
    am = np.asarray(attention_mask).astype(bool)
    budget = int(block_budget)
    assert budget % 8 == 0 and 0 < budget <= NB
    n_rounds = budget // 8

    scale = 1.0 / math.sqrt(DG)

    key = (n_rounds,)
    if key not in _compiled:
        _compiled[key] = _build_program(n_rounds)
    nc = _compiled[key]

    idn_np = np.eye(128, dtype=np.float32)
    ones_np = np.ones((128, 1), dtype=np.float32)

    in_maps = []
    for c in range(N_CORES):
        b = c // 2
        h0 = (c % 2) * HEADS_PER_CORE
        heads = list(range(h0, h0 + HEADS_PER_CORE))
        im = {}
        for i, h in enumerate(heads):
            im["k%d" % i] = np.ascontiguousarray(k[b, :, h, :])
        # wk: [d, h, t(mean/max), o]; mean part scaled by 1/64
        wk_prep = np.empty((D, HEADS_PER_CORE, 2, DG), dtype=np.float32)
        for i, h in enumerate(heads):
            wk_prep[:, i, 0, :] = Wk[h, :D, :] / BLOCK
            wk_prep[:, i, 1, :] = Wk[h, D:, :]
        im["wk"] = wk_prep
        # wq: [d, (h, g, o)] with contraction index i=(g,d) split as d-partition
        wq_prep = np.empty((D, HEADS_PER_CORE, G, DG), dtype=np.float32)
        for i, h in enumerate(heads):
            wq_prep[:, i, :, :] = Wq[h].reshape(G, D, DG).transpose(1, 0, 2)
        im["wq"] = wq_prep.reshape(D, HEADS_PER_CORE * G * DG)
        # qvec: [d, (h, g)]
        qv_prep = np.empty((D, HEADS_PER_CORE, G), dtype=np.float32)
        for i, h in enumerate(heads):
            qv_prep[:, i, :] = q[b, 0, h * G:(h + 1) * G, :].T
        im["qvec"] = qv_prep.reshape(D, HEADS_PER_CORE * G)
        # folded cos/sin (q): carry qnorm_w, rotation sign and the 1/sqrt(Dg)
        cqv = cos_q[b, 0] * qnorm_w * scale
        sqv = sin_q[b, 0] * _rot_w(qnorm_w) * scale
        sqv = sqv.copy()
        sqv[:DG // 2] *= -1.0
        im["cq"] = np.tile(cqv, (HEADS_PER_CORE, 1)).astype(np.float32)
        im["sq"] = np.tile(sqv, (HEADS_PER_CORE, 1)).astype(np.float32)
        # folded cos/sin (k)
        ckv = cos_k[b] * knorm_w[None, :]
        skv = sin_k[b] * _rot_w(knorm_w)[None, :]
        skv = skv.copy()
        skv[:, :DG // 2] *= -1.0
        im["ck"] = ckv.astype(np.float32)
        im["sk"] = skv.astype(np.float32)
        im["amask"] = am[b, heads, :].astype(np.float32)
        im["idn"] = idn_np
        im["ones_col"] = ones_np
        in_maps.append(im)

    res = run_bass_kernel_spmd(nc, in_maps, core_ids=list(range(N_CORES)),
                               trace=bool(int(os.environ.get("ATTNGATE_TRACE", "0"))))
    kernel.last_result = res

    sel = np.zeros((B, HK, NB), dtype=bool)
    for c in range(N_CORES):
        b = c // 2
        h0 = (c % 2) * HEADS_PER_CORE
        sel[b, h0:h0 + HEADS_PER_CORE, :] = res.results[c]["out_mask"] != 0.0
    mask = sel & am
    mask[:, :, -1] = True
    return mask
